# revision 23
# baseline (speedup 1.0000x reference)
"""Causal GQA attention on 8 TRN2 NeuronCores.

Problem: q [2048, 32, 128] f32, k/v [2048, 8, 128] f32, causal attention
with 4 query heads per kv head (GQA). Sharding: tensor-parallel over kv
heads -- core i gets kv head i plus query heads 4i..4i+3. No cross-core
communication needed.

Per-core algorithm (T=S=2048, HQ=4 local q heads, D=128):
  * Q/K/V loaded f32 in fine-grained groups ordered so DMA bandwidth
    feeds the first chunks first; cast to fp16 on DVE. K/Q transposed to
    [d, s]/[d, t] layout by PE identity-transposes emitted on demand one
    chunk ahead (PSUM staging tiles borrow the score pool's slots).
  * Scores are computed TRANSPOSED: st[s_block=128, q_chunk<=512] =
    K_b^T-stationary x Q^T-moving; fp32 PSUM. Per-block causal trim
    (each block of a pair starts at its own diagonal offset; the skipped
    region of the second block is never read downstream).
  * Softmax exp is SPLIT across two engines (exp on ScalarE alone is an
    82us serial bottleneck at 1 elem/cycle/lane):
      - 3/4 of block-pairs: ScalarE LUT exp (exact, ~2ulp), fp16 out.
      - 1/4 of block-pairs: DVE Schraudolph exp -- one tensor_scalar
        i16 = rne(score * SCALE*log2e*1024 + (15360-57.75)) whose int16
        bit pattern IS fp16 2^t with linearly-interpolated mantissa.
        The 57.75 bias centers the mean ratio at 1 so mixing with exact
        exp stays unbiased. Measured end-to-end rel err 4.3e-3 (gate
        2e-2); the fraction is a speed/accuracy dial (1.4e-2 at 100%).
  * Causal mask: GPSIMD affine_select zeroes the s>q triangle of the
    diagonal prob tiles after exp.
  * PV: prob block [s,q-tile] is the STATIONARY operand, moving operand
    is [V_b | ones] [s, 129] fp16: accumulates [q, 128 out + 1 denom]
    in PSUM over s blocks -- the softmax denominator comes for free.
  * Software pipeline: QK runs TWO pairs ahead of PV (sc_psum bufs=3,
    6 banks) so the cross-engine exp latency is hidden; exp is emitted
    before the previous pair's PV/finalize so DVE exps don't queue
    behind finalize muls in the in-order DVE FIFO.
  * PV accumulators pack 2 tlocs per PSUM bank; only the first MM
    touching a bank carries start=True (start clears the WHOLE bank,
    per-element has_written handles overwrite-vs-accumulate after).
  * Finalize: DVE reciprocal of denom pairs + per-partition scalar
    multiply to fp32 SBUF, DMA out per chunk.
"""

import math

import numpy as np

import concourse.bass as bass
import concourse.tile as tile
from concourse import bacc, mybir
from concourse.masks import make_identity

P = 128
F32 = mybir.dt.float32
F16 = mybir.dt.float16
I16 = mybir.dt.int16
EXP = mybir.ActivationFunctionType.Exp

T_FULL = 2048
S_FULL = 2048
NH = 32
NKV = 8
D = 128
HQ = NH // NKV  # q heads per kv head (= per core)
N_CORES = 8

# Schraudolph fraction (pairs sent to DVE) and bias constant.
DVE_NUM, DVE_DEN = 1, 4
SCH_C = 57.75


def _attention_body(tc, T, S, HQ, D, chunk):
    nc = tc.nc
    NT = T // P          # q tiles
    NB = S // P          # s blocks
    TPC = chunk // P     # q tiles per chunk
    NCH = T // chunk     # chunks
    assert TPC % 2 == 0 and T % chunk == 0 and S == T
    SCALE = 1.0 / math.sqrt(D)
    LOG2E = math.log2(math.e)
    A_SCH = SCALE * LOG2E * 1024.0
    B_SCH = 15360.0 - SCH_C

    q = nc.dram_tensor("q", [T, HQ, D], F32, kind="ExternalInput").ap()
    k = nc.dram_tensor("k", [S, D], F32, kind="ExternalInput").ap()
    v = nc.dram_tensor("v", [S, D], F32, kind="ExternalInput").ap()
    out = nc.dram_tensor("out", [T, HQ, D], F32, kind="ExternalOutput").ap()

    from contextlib import ExitStack

    with ExitStack() as ctx:
        consts = ctx.enter_context(tc.tile_pool(name="consts", bufs=1))
        qT_pool = ctx.enter_context(tc.tile_pool(name="qT", bufs=4))
        q32_pool = ctx.enter_context(tc.tile_pool(name="q32", bufs=3))
        q16_pool = ctx.enter_context(tc.tile_pool(name="q16", bufs=3))
        et_pool = ctx.enter_context(tc.tile_pool(name="et", bufs=8))
        osb_pool = ctx.enter_context(tc.tile_pool(name="osb", bufs=4))
        rec_pool = ctx.enter_context(tc.tile_pool(name="rec", bufs=8))
        sc_psum = ctx.enter_context(tc.tile_pool(name="sc", bufs=3, space="PSUM"))
        pv_psum = ctx.enter_context(tc.tile_pool(name="pv", bufs=2, space="PSUM"))

        ident = consts.tile([P, P], F16)

        # ---- startup loads: dispatch everything with no deps first ----
        k_nat32 = consts.tile([P, NB, P], F32)
        k_nat = consts.tile([P, NB, P], F16)
        kT = consts.tile([P, NB * P], F16)
        k_r = k.rearrange("(b p) d -> p b d", p=P)
        v_sb = consts.tile([P, NB, P + 1], F16)  # [s_in_block, b, d|ones]
        v_nat32 = consts.tile([P, NB, P], F32)
        v_r = v.rearrange("(b p) d -> p b d", p=P)

        qTs = {}
        q32s = {}
        q16s = {}

        def q_tiles(h):
            if h not in qTs:
                qTs[h] = qT_pool.tile([P, T], F16, name=f"qT{h}", tag="qT")
                q32s[h] = q32_pool.tile([P, NT, P], F32, name=f"q32_{h}", tag="q32")
                q16s[h] = q16_pool.tile([P, NT, P], F16, name=f"q16_{h}", tag="q16")
            return qTs[h], q32s[h], q16s[h]

        def emit_q_load(h, g0, g1):
            # load + cast q tiles [g0*4, g1*4) of head h
            _, q32, q16 = q_tiles(h)
            t0, t1 = 4 * g0, 4 * g1
            q_rh = q[:, h, :].rearrange("(t p) d -> p t d", p=P)
            nc.sync.dma_start(out=q32[:, t0:t1, :], in_=q_rh[:, t0:t1, :])
            nc.vector.tensor_copy(q16[:, t0:t1, :], q32[:, t0:t1, :])

        q_loaded = set()

        def emit_q_load_full(h):
            if h in q_loaded:
                return
            q_loaded.add(h)
            q_tiles(h)
            emit_q_load(h, 0, 2)
            emit_q_load(h, 2, 4)

        # startup loads in NEED order, fine-grained so DMA bandwidth goes
        # to the first chunks' data first: chunk (0,c) needs k/q0 group c
        # and PV needs v blocks progressively.
        def k_load(g):
            bg = 4 * g
            nc.sync.dma_start(out=k_nat32[:, bg : bg + 4, :], in_=k_r[:, bg : bg + 4, :])
            nc.vector.tensor_copy(k_nat[:, bg : bg + 4, :], k_nat32[:, bg : bg + 4, :])

        def v_load(g):
            bg = 4 * g
            nc.sync.dma_start(out=v_nat32[:, bg : bg + 4, :], in_=v_r[:, bg : bg + 4, :])
            nc.vector.tensor_copy(v_sb[:, bg : bg + 4, 0:P], v_nat32[:, bg : bg + 4, :])

        k_load(0)
        emit_q_load(0, 0, 1)
        make_identity(nc, ident)
        v_load(0)
        emit_q_load(0, 1, 2)
        k_load(1)
        v_load(1)
        nc.vector.memset(v_sb[:, :, P : P + 1], 1.0)
        emit_q_load(0, 2, 3)
        k_load(2)
        v_load(2)
        emit_q_load(0, 3, 4)
        k_load(3)
        v_load(3)

        # ---- PE identity-transpose path (K and head 0) ----
        def pe_transpose(dst, src_tiles):
            # src_tiles: list of [P, P] f16 APs; dst: [P, 4*P] slice of kT/qT
            tp = sc_psum.tile([P, 4 * P], F16, tag="sc")
            for j, s_ap in enumerate(src_tiles):
                nc.tensor.transpose(tp[:, j * P : (j + 1) * P], s_ap, ident)
            nc.vector.tensor_copy(dst, tp)

        k_tp_done = set()

        def emit_ktp(g):
            if g in k_tp_done:
                return
            k_tp_done.add(g)
            bg = 4 * g
            pe_transpose(
                kT[:, bg * P : (bg + 4) * P],
                [k_nat[:, bg + j, :] for j in range(4)],
            )

        q_tp_done = set()

        def emit_qtp(h, c):
            if (h, c) in q_tp_done:
                return
            q_tp_done.add((h, c))
            qT, _, q16 = q_tiles(h)
            pe_transpose(
                qT[:, c * chunk : (c + 1) * chunk],
                [q16[:, c * TPC + j, :] for j in range(TPC)],
            )

        emit_ktp(0)
        emit_qtp(0, 0)

        # head 1 loads now (after startup loads); transposes on demand.
        emit_q_load_full(1)

        schedule = []
        for h in range(HQ):
            for cc in range(NCH):
                schedule.append((h, cc))

        # Bresenham assignment of pairs to the DVE Schraudolph path
        dve_pairs = set()
        acc = 0
        for h in range(HQ):
            for cc in range(NCH):
                for b0 in range(0, TPC * (cc + 1), 2):
                    acc += DVE_NUM
                    if acc >= DVE_DEN:
                        acc -= DVE_DEN
                        dve_pairs.add((h, cc, b0))

        def emit_prefetch(idx):
            h, c = schedule[idx]
            if c + 1 < NCH:
                if h == 0:
                    emit_ktp(c + 1)
                emit_qtp(h, c + 1)
            elif h + 1 < HQ:
                emit_qtp(h + 1, 0)
            if c == 0 and h + 2 < HQ:
                emit_q_load_full(h + 2)

        chunk_state = {}

        def get_state(idx, h, c):
            if idx not in chunk_state:
                chunk_state[idx] = {
                    "pvs": [
                        pv_psum.tile(
                            [P, 2, 132], F32, name=f"pv{idx}_{g}", tag="pv"
                        )
                        for g in range(TPC // 2)
                    ],
                    "osb": osb_pool.tile(
                        [P, TPC, P], F32, name=f"osb{idx}", tag="osb"
                    ),
                    "started": set(),
                }
            return chunk_state[idx]

        def emit_qk(idx, h, c, b0):
            qT = qTs[h]
            sc = sc_psum.tile([P, 2 * chunk], F32, name=f"sc{idx}_{b0}", tag="sc")
            for i, b in enumerate((b0, b0 + 1)):
                joff = max(0, b - c * TPC) * P
                nc.tensor.matmul(
                    sc[:, i * chunk + joff : (i + 1) * chunk],
                    lhsT=kT[:, b * P : (b + 1) * P],
                    rhs=qT[:, c * chunk + joff : (c + 1) * chunk],
                    start=True,
                    stop=True,
                )
            return sc

        def emit_exp_mask(idx, h, c, b0, sc):
            pair = (b0, b0 + 1)
            et = et_pool.tile([P, 2 * chunk], F16, name=f"et{idx}_{b0}", tag="et")
            joff0 = max(0, b0 - c * TPC) * P
            rng = slice(joff0, 2 * chunk)
            if (h, c, b0) in dve_pairs:
                nc.vector.tensor_scalar(
                    out=et[:, rng].bitcast(I16),
                    in0=sc[:, rng],
                    scalar1=A_SCH,
                    scalar2=B_SCH,
                    op0=mybir.AluOpType.mult,
                    op1=mybir.AluOpType.add,
                )
            else:
                nc.scalar.activation(et[:, rng], sc[:, rng], EXP, scale=SCALE)
            if b0 >= c * TPC:
                for i, b in enumerate(pair):
                    j = b - c * TPC
                    dsl = et[:, i * chunk + j * P : i * chunk + (j + 1) * P]
                    nc.gpsimd.affine_select(
                        out=dsl,
                        in_=dsl,
                        pattern=[[1, P]],
                        compare_op=mybir.AluOpType.is_ge,
                        fill=0.0,
                        base=0,
                        channel_multiplier=-1,
                    )
            return et

        def emit_pv(idx, h, c, b0, et):
            st = get_state(idx, h, c)
            work = []
            for i, b in enumerate((b0, b0 + 1)):
                j = b - c * TPC
                for tloc in range(max(0, j), TPC):
                    work.append((i, b, tloc, tloc == j))
            if b0 == 0:
                # first pair of a chunk: run bank-group-0 tlocs first --
                # group 1's bank is freed by the previous chunk's LAST
                # finalize, so giving it ~4 MMs of slack hides the handoff
                work.sort(key=lambda w: (w[3], w[2] // 2, w[0]))
            else:
                work.sort(key=lambda w: w[3])  # diagonal-tile PV last
            for i, b, tloc, _ in work:
                t = c * TPC + tloc
                g = tloc // 2
                first_touch = g not in st["started"]
                st["started"].add(g)
                nc.tensor.matmul(
                    st["pvs"][g][:, tloc % 2, 0 : P + 1],
                    lhsT=et[:, i * chunk + tloc * P : i * chunk + (tloc + 1) * P],
                    rhs=v_sb[:, b, :],
                    start=first_touch,
                    stop=(b == t),
                )

        def emit_finalize(idx, h, c, b0):
            st = chunk_state[idx]
            for b in (b0, b0 + 1):
                tloc = b - c * TPC
                if tloc < 0 or tloc % 2 != 0:
                    continue
                g = tloc // 2
                pv = st["pvs"][g]
                rec = rec_pool.tile(
                    [P, 2, 1], F32, name=f"rec{idx}_{g}", tag="rec"
                )
                nc.vector.reciprocal(rec, pv[:, :, P : P + 1])
                for j in range(2):
                    nc.vector.tensor_scalar_mul(
                        st["osb"][:, 2 * g + j, :], pv[:, j, 0:P], rec[:, j, :]
                    )

        def flush(entry):
            idx, h, c, b0, last, et = entry
            emit_pv(idx, h, c, b0, et)
            emit_finalize(idx, h, c, b0)
            if b0 == 0:
                emit_prefetch(idx)
            if last:
                nc.sync.dma_start(
                    out=out[c * chunk : (c + 1) * chunk, h, :].rearrange(
                        "(t p) d -> p t d", p=P
                    ),
                    in_=chunk_state[idx]["osb"],
                )
                del chunk_state[idx]

        # one flat software-pipelined stream over every (chunk, pair)
        stream = []
        for idx, (h, c) in enumerate(schedule):
            nblocks = TPC * (c + 1)
            for b0 in range(0, nblocks, 2):
                stream.append((idx, h, c, b0, b0 == nblocks - 2))

        # QK runs TWO pairs ahead of PV; exp emitted before the previous
        # pair's flush so DVE exps don't queue behind finalize muls.
        scs = {}

        def emit_qk_for(j):
            idx, h, c, b0, last = stream[j]
            # safety net for short chunks: the flush-prefetch may not
            # have emitted the PE transposes this QK reads yet
            for g in range((b0 + 1) // 4 + 1):
                emit_ktp(g)
            emit_qtp(h, c)
            get_state(idx, h, c)
            scs[j] = emit_qk(idx, h, c, b0)

        emit_qk_for(0)
        emit_qk_for(1)
        prev = None
        for j, (idx, h, c, b0, last) in enumerate(stream):
            if j + 2 < len(stream):
                emit_qk_for(j + 2)
            et = emit_exp_mask(idx, h, c, b0, scs.pop(j))
            if prev is not None:
                flush(prev)
            prev = (idx, h, c, b0, last, et)
        flush(prev)


def build_nc(T=T_FULL, S=S_FULL, HQ=HQ, D=D, chunk=512):
    nc = bacc.Bacc(
        "TRN2", target_bir_lowering=False, debug=False, enable_asserts=False
    )
    with tile.TileContext(nc) as tc:
        _attention_body(tc, T, S, HQ, D, chunk)
    nc.compile()
    return nc


_NC_CACHE = {}


def _get_nc():
    if "nc" not in _NC_CACHE:
        _NC_CACHE["nc"] = build_nc()
    return _NC_CACHE["nc"]


def kernel(q, k, v):
    """Full-problem entry point: q [2048,32,128], k/v [2048,8,128] f32."""
    from concourse.bass_utils import run_bass_kernel_spmd

    q = np.asarray(q, dtype=np.float32)
    k = np.asarray(k, dtype=np.float32)
    v = np.asarray(v, dtype=np.float32)

    nc = _get_nc()
    in_maps = []
    for i in range(N_CORES):
        in_maps.append(
            {
                "q": np.ascontiguousarray(q[:, HQ * i : HQ * (i + 1), :]),
                "k": np.ascontiguousarray(k[:, i, :]),
                "v": np.ascontiguousarray(v[:, i, :]),
            }
        )
    res = run_bass_kernel_spmd(nc, in_maps, core_ids=list(range(N_CORES)))
    out = np.empty((T_FULL, NH, D), dtype=np.float32)
    for i in range(N_CORES):
        out[:, HQ * i : HQ * (i + 1), :] = res.results[i]["out"]
    return out


# revision 24
# speedup vs baseline: 1.0706x; 1.0706x over previous
"""Causal GQA attention on 8 TRN2 NeuronCores.

Problem: q [2048, 32, 128] f32, k/v [2048, 8, 128] f32, causal attention
with 4 query heads per kv head (GQA). Sharding: tensor-parallel over kv
heads -- core i gets kv head i plus query heads 4i..4i+3. No cross-core
communication needed.

Per-core algorithm (T=S=2048, HQ=4 local q heads, D=128):
  * Q/K/V loaded f32 in fine-grained groups ordered so DMA bandwidth
    feeds the first chunks first; cast to fp16 on DVE. K/Q transposed to
    [d, s]/[d, t] layout by PE identity-transposes emitted on demand one
    chunk ahead (PSUM staging tiles borrow the score pool's slots).
  * Scores are computed TRANSPOSED: st[s_block=128, q_chunk<=512] =
    K_b^T-stationary x Q^T-moving; fp32 PSUM. Per-block causal trim
    (each block of a pair starts at its own diagonal offset; the skipped
    region of the second block is never read downstream).
  * Softmax exp is SPLIT across two engines (exp on ScalarE alone is an
    82us serial bottleneck at 1 elem/cycle/lane):
      - 3/4 of block-pairs: ScalarE LUT exp (exact, ~2ulp), fp16 out.
      - 1/4 of block-pairs: DVE Schraudolph exp -- one tensor_scalar
        i16 = rne(score * SCALE*log2e*1024 + (15360-57.75)) whose int16
        bit pattern IS fp16 2^t with linearly-interpolated mantissa.
        The 57.75 bias centers the mean ratio at 1 so mixing with exact
        exp stays unbiased. Measured end-to-end rel err 4.3e-3 (gate
        2e-2); the fraction is a speed/accuracy dial (1.4e-2 at 100%).
  * Causal mask: GPSIMD affine_select zeroes the s>q triangle of the
    diagonal prob tiles after exp.
  * PV: prob block [s,q-tile] is the STATIONARY operand, moving operand
    is [V_b | ones] [s, 129] fp16: accumulates [q, 128 out + 1 denom]
    in PSUM over s blocks -- the softmax denominator comes for free.
  * Software pipeline: QK runs TWO pairs ahead of PV (sc_psum bufs=3,
    6 banks) so the cross-engine exp latency is hidden; exp is emitted
    before the previous pair's PV/finalize so DVE exps don't queue
    behind finalize muls in the in-order DVE FIFO.
  * PV accumulators pack 2 tlocs per PSUM bank; only the first MM
    touching a bank carries start=True (start clears the WHOLE bank,
    per-element has_written handles overwrite-vs-accumulate after).
  * Finalize: DVE reciprocal of denom pairs + per-partition scalar
    multiply to fp32 SBUF, DMA out per chunk.
"""

import math

import numpy as np

import concourse.bass as bass
import concourse.tile as tile
from concourse import bacc, mybir
from concourse.masks import make_identity

P = 128
F32 = mybir.dt.float32
F16 = mybir.dt.float16
I16 = mybir.dt.int16
EXP = mybir.ActivationFunctionType.Exp

T_FULL = 2048
S_FULL = 2048
NH = 32
NKV = 8
D = 128
HQ = NH // NKV  # q heads per kv head (= per core)
N_CORES = 8

# Schraudolph fraction (pairs sent to DVE) and bias constant.
DVE_NUM, DVE_DEN = 1, 4
SCH_C = 57.75


def _attention_body(tc, T, S, HQ, D, chunk):
    nc = tc.nc
    NT = T // P          # q tiles
    NB = S // P          # s blocks
    TPC = chunk // P     # q tiles per chunk
    NCH = T // chunk     # chunks
    assert TPC % 2 == 0 and T % chunk == 0 and S == T
    SCALE = 1.0 / math.sqrt(D)
    LOG2E = math.log2(math.e)
    A_SCH = SCALE * LOG2E * 1024.0
    B_SCH = 15360.0 - SCH_C

    q = nc.dram_tensor("q", [T, HQ, D], F32, kind="ExternalInput").ap()
    k = nc.dram_tensor("k", [S, D], F32, kind="ExternalInput").ap()
    v = nc.dram_tensor("v", [S, D], F32, kind="ExternalInput").ap()
    out = nc.dram_tensor("out", [T, HQ, D], F32, kind="ExternalOutput").ap()

    from contextlib import ExitStack

    with ExitStack() as ctx:
        consts = ctx.enter_context(tc.tile_pool(name="consts", bufs=1))
        qT_pool = ctx.enter_context(tc.tile_pool(name="qT", bufs=4))
        q32_pool = ctx.enter_context(tc.tile_pool(name="q32", bufs=3))
        q16_pool = ctx.enter_context(tc.tile_pool(name="q16", bufs=3))
        et_pool = ctx.enter_context(tc.tile_pool(name="et", bufs=8))
        osb_pool = ctx.enter_context(tc.tile_pool(name="osb", bufs=4))
        rec_pool = ctx.enter_context(tc.tile_pool(name="rec", bufs=8))
        sc_psum = ctx.enter_context(tc.tile_pool(name="sc", bufs=3, space="PSUM"))
        pv_psum = ctx.enter_context(tc.tile_pool(name="pv", bufs=2, space="PSUM"))

        ident = consts.tile([P, P], F16)

        # ---- startup loads: dispatch everything with no deps first ----
        k_nat32 = consts.tile([P, NB, P], F32)
        k_nat = consts.tile([P, NB, P], F16)
        kT = consts.tile([P, NB * P], F16)
        k_r = k.rearrange("(b p) d -> p b d", p=P)
        v_sb = consts.tile([P, NB, P + 1], F16)  # [s_in_block, b, d|ones]
        v_nat32 = consts.tile([P, NB, P], F32)
        v_r = v.rearrange("(b p) d -> p b d", p=P)

        qTs = {}
        q32s = {}
        q16s = {}

        def q_tiles(h):
            if h not in qTs:
                qTs[h] = qT_pool.tile([P, T], F16, name=f"qT{h}", tag="qT")
                q32s[h] = q32_pool.tile([P, NT, P], F32, name=f"q32_{h}", tag="q32")
                q16s[h] = q16_pool.tile([P, NT, P], F16, name=f"q16_{h}", tag="q16")
            return qTs[h], q32s[h], q16s[h]

        def emit_q_load(h, g0, g1):
            # load + cast q tiles [g0*4, g1*4) of head h
            _, q32, q16 = q_tiles(h)
            t0, t1 = 4 * g0, 4 * g1
            q_rh = q[:, h, :].rearrange("(t p) d -> p t d", p=P)
            nc.sync.dma_start(out=q32[:, t0:t1, :], in_=q_rh[:, t0:t1, :])
            nc.vector.tensor_copy(q16[:, t0:t1, :], q32[:, t0:t1, :])

        q_loaded = set()

        def emit_q_load_full(h):
            if h in q_loaded:
                return
            q_loaded.add(h)
            q_tiles(h)
            emit_q_load(h, 0, 2)
            emit_q_load(h, 2, 4)

        # startup loads in NEED order, fine-grained so DMA bandwidth goes
        # to the first chunks' data first: chunk (0,c) needs k/q0 group c
        # and PV needs v blocks progressively.
        def k_load(g):
            bg = 4 * g
            nc.sync.dma_start(out=k_nat32[:, bg : bg + 4, :], in_=k_r[:, bg : bg + 4, :])
            nc.vector.tensor_copy(k_nat[:, bg : bg + 4, :], k_nat32[:, bg : bg + 4, :])

        def v_load(g):
            bg = 4 * g
            nc.sync.dma_start(out=v_nat32[:, bg : bg + 4, :], in_=v_r[:, bg : bg + 4, :])
            nc.vector.tensor_copy(v_sb[:, bg : bg + 4, 0:P], v_nat32[:, bg : bg + 4, :])

        k_load(0)
        emit_q_load(0, 0, 1)
        make_identity(nc, ident)
        v_load(0)
        emit_q_load(0, 1, 2)
        k_load(1)
        v_load(1)
        nc.vector.memset(v_sb[:, :, P : P + 1], 1.0)
        emit_q_load(0, 2, 3)
        k_load(2)
        v_load(2)
        emit_q_load(0, 3, 4)
        k_load(3)
        v_load(3)

        # ---- PE warmup: dummy transposes while the first loads are in
        # flight trip the HAM activity window so the first real matmuls
        # run at 2.4GHz instead of the cold 1.2GHz half-rate. Sized to
        # finish right as the first cast lands (no queue delay).
        warm_tp = sc_psum.tile([P, 4 * P], F16, tag="sc")
        for i in range(16):
            nc.tensor.transpose(
                warm_tp[:, (i % 4) * P : (i % 4 + 1) * P], ident, ident
            )

        # ---- PE identity-transpose path (K and head 0) ----
        def pe_transpose(dst, src_tiles):
            # src_tiles: list of [P, P] f16 APs; dst: [P, 4*P] slice of kT/qT
            tp = sc_psum.tile([P, 4 * P], F16, tag="sc")
            for j, s_ap in enumerate(src_tiles):
                nc.tensor.transpose(tp[:, j * P : (j + 1) * P], s_ap, ident)
            nc.vector.tensor_copy(dst, tp)

        k_tp_done = set()

        def emit_ktp(g):
            if g in k_tp_done:
                return
            k_tp_done.add(g)
            bg = 4 * g
            pe_transpose(
                kT[:, bg * P : (bg + 4) * P],
                [k_nat[:, bg + j, :] for j in range(4)],
            )

        q_tp_done = set()

        def emit_qtp(h, c):
            if (h, c) in q_tp_done:
                return
            q_tp_done.add((h, c))
            qT, _, q16 = q_tiles(h)
            pe_transpose(
                qT[:, c * chunk : (c + 1) * chunk],
                [q16[:, c * TPC + j, :] for j in range(TPC)],
            )

        emit_ktp(0)
        emit_qtp(0, 0)

        # head 1 loads now (after startup loads); transposes on demand.
        emit_q_load_full(1)

        schedule = []
        for h in range(HQ):
            for cc in range(NCH):
                schedule.append((h, cc))

        # Bresenham assignment of pairs to the DVE Schraudolph path
        dve_pairs = set()
        acc = 0
        for h in range(HQ):
            for cc in range(NCH):
                for b0 in range(0, TPC * (cc + 1), 2):
                    acc += DVE_NUM
                    if acc >= DVE_DEN:
                        acc -= DVE_DEN
                        dve_pairs.add((h, cc, b0))

        def emit_prefetch(idx):
            h, c = schedule[idx]
            if c + 1 < NCH:
                if h == 0:
                    emit_ktp(c + 1)
                emit_qtp(h, c + 1)
            elif h + 1 < HQ:
                emit_qtp(h + 1, 0)
            if c == 0 and h + 2 < HQ:
                emit_q_load_full(h + 2)

        chunk_state = {}

        def get_state(idx, h, c):
            if idx not in chunk_state:
                chunk_state[idx] = {
                    "pvs": [
                        pv_psum.tile(
                            [P, 2, 132], F32, name=f"pv{idx}_{g}", tag="pv"
                        )
                        for g in range(TPC // 2)
                    ],
                    "osb": osb_pool.tile(
                        [P, TPC, P], F32, name=f"osb{idx}", tag="osb"
                    ),
                    "started": set(),
                }
            return chunk_state[idx]

        def emit_qk(idx, h, c, b0):
            qT = qTs[h]
            sc = sc_psum.tile([P, 2 * chunk], F32, name=f"sc{idx}_{b0}", tag="sc")
            for i, b in enumerate((b0, b0 + 1)):
                joff = max(0, b - c * TPC) * P
                nc.tensor.matmul(
                    sc[:, i * chunk + joff : (i + 1) * chunk],
                    lhsT=kT[:, b * P : (b + 1) * P],
                    rhs=qT[:, c * chunk + joff : (c + 1) * chunk],
                    start=True,
                    stop=True,
                )
            return sc

        def emit_exp_mask(idx, h, c, b0, sc):
            pair = (b0, b0 + 1)
            et = et_pool.tile([P, 2 * chunk], F16, name=f"et{idx}_{b0}", tag="et")
            joff0 = max(0, b0 - c * TPC) * P
            rng = slice(joff0, 2 * chunk)
            if (h, c, b0) in dve_pairs:
                nc.vector.tensor_scalar(
                    out=et[:, rng].bitcast(I16),
                    in0=sc[:, rng],
                    scalar1=A_SCH,
                    scalar2=B_SCH,
                    op0=mybir.AluOpType.mult,
                    op1=mybir.AluOpType.add,
                )
            else:
                nc.scalar.activation(et[:, rng], sc[:, rng], EXP, scale=SCALE)
            if b0 >= c * TPC:
                for i, b in enumerate(pair):
                    j = b - c * TPC
                    dsl = et[:, i * chunk + j * P : i * chunk + (j + 1) * P]
                    nc.gpsimd.affine_select(
                        out=dsl,
                        in_=dsl,
                        pattern=[[1, P]],
                        compare_op=mybir.AluOpType.is_ge,
                        fill=0.0,
                        base=0,
                        channel_multiplier=-1,
                    )
            return et

        def emit_pv(idx, h, c, b0, et):
            st = get_state(idx, h, c)
            work = []
            for i, b in enumerate((b0, b0 + 1)):
                j = b - c * TPC
                for tloc in range(max(0, j), TPC):
                    work.append((i, b, tloc, tloc == j))
            if b0 == 0:
                # first pair of a chunk: run bank-group-0 tlocs first --
                # group 1's bank is freed by the previous chunk's LAST
                # finalize, so giving it ~4 MMs of slack hides the handoff
                work.sort(key=lambda w: (w[3], w[2] // 2, w[0]))
            else:
                work.sort(key=lambda w: w[3])  # diagonal-tile PV last
            for i, b, tloc, _ in work:
                t = c * TPC + tloc
                g = tloc // 2
                first_touch = g not in st["started"]
                st["started"].add(g)
                nc.tensor.matmul(
                    st["pvs"][g][:, tloc % 2, 0 : P + 1],
                    lhsT=et[:, i * chunk + tloc * P : i * chunk + (tloc + 1) * P],
                    rhs=v_sb[:, b, :],
                    start=first_touch,
                    stop=(b == t),
                )

        def emit_finalize(idx, h, c, b0):
            st = chunk_state[idx]
            for b in (b0, b0 + 1):
                tloc = b - c * TPC
                if tloc < 0 or tloc % 2 != 0:
                    continue
                g = tloc // 2
                pv = st["pvs"][g]
                rec = rec_pool.tile(
                    [P, 2, 1], F32, name=f"rec{idx}_{g}", tag="rec"
                )
                nc.vector.reciprocal(rec, pv[:, :, P : P + 1])
                for j in range(2):
                    nc.vector.tensor_scalar_mul(
                        st["osb"][:, 2 * g + j, :], pv[:, j, 0:P], rec[:, j, :]
                    )

        def flush(entry):
            idx, h, c, b0, last, et = entry
            emit_pv(idx, h, c, b0, et)
            emit_finalize(idx, h, c, b0)
            if b0 == 0:
                emit_prefetch(idx)
            if last:
                nc.sync.dma_start(
                    out=out[c * chunk : (c + 1) * chunk, h, :].rearrange(
                        "(t p) d -> p t d", p=P
                    ),
                    in_=chunk_state[idx]["osb"],
                )
                del chunk_state[idx]

        # one flat software-pipelined stream over every (chunk, pair)
        stream = []
        for idx, (h, c) in enumerate(schedule):
            nblocks = TPC * (c + 1)
            for b0 in range(0, nblocks, 2):
                stream.append((idx, h, c, b0, b0 == nblocks - 2))

        # QK runs TWO pairs ahead of PV; exp emitted before the previous
        # pair's flush so DVE exps don't queue behind finalize muls.
        scs = {}

        def emit_qk_for(j):
            idx, h, c, b0, last = stream[j]
            # safety net for short chunks: the flush-prefetch may not
            # have emitted the PE transposes this QK reads yet
            for g in range((b0 + 1) // 4 + 1):
                emit_ktp(g)
            emit_qtp(h, c)
            get_state(idx, h, c)
            scs[j] = emit_qk(idx, h, c, b0)

        emit_qk_for(0)
        emit_qk_for(1)
        prev = None
        for j, (idx, h, c, b0, last) in enumerate(stream):
            if j + 2 < len(stream):
                emit_qk_for(j + 2)
            et = emit_exp_mask(idx, h, c, b0, scs.pop(j))
            if prev is not None:
                flush(prev)
            prev = (idx, h, c, b0, last, et)
        flush(prev)


def build_nc(T=T_FULL, S=S_FULL, HQ=HQ, D=D, chunk=512):
    nc = bacc.Bacc(
        "TRN2", target_bir_lowering=False, debug=False, enable_asserts=False
    )
    with tile.TileContext(nc) as tc:
        _attention_body(tc, T, S, HQ, D, chunk)
    nc.compile()
    return nc


_NC_CACHE = {}


def _get_nc():
    if "nc" not in _NC_CACHE:
        _NC_CACHE["nc"] = build_nc()
    return _NC_CACHE["nc"]


def kernel(q, k, v):
    """Full-problem entry point: q [2048,32,128], k/v [2048,8,128] f32."""
    from concourse.bass_utils import run_bass_kernel_spmd

    q = np.asarray(q, dtype=np.float32)
    k = np.asarray(k, dtype=np.float32)
    v = np.asarray(v, dtype=np.float32)

    nc = _get_nc()
    in_maps = []
    for i in range(N_CORES):
        in_maps.append(
            {
                "q": np.ascontiguousarray(q[:, HQ * i : HQ * (i + 1), :]),
                "k": np.ascontiguousarray(k[:, i, :]),
                "v": np.ascontiguousarray(v[:, i, :]),
            }
        )
    res = run_bass_kernel_spmd(nc, in_maps, core_ids=list(range(N_CORES)))
    out = np.empty((T_FULL, NH, D), dtype=np.float32)
    for i in range(N_CORES):
        out[:, HQ * i : HQ * (i + 1), :] = res.results[i]["out"]
    return out


# revision 25
# speedup vs baseline: 1.0773x; 1.0063x over previous
"""Causal GQA attention on 8 TRN2 NeuronCores.

Problem: q [2048, 32, 128] f32, k/v [2048, 8, 128] f32, causal attention
with 4 query heads per kv head (GQA). Sharding: tensor-parallel over kv
heads -- core i gets kv head i plus query heads 4i..4i+3. No cross-core
communication needed.

Per-core algorithm (T=S=2048, HQ=4 local q heads, D=128):
  * Q/K/V loaded f32 in fine-grained groups ordered so DMA bandwidth
    feeds the first chunks first; cast to fp16 on DVE. K/Q transposed to
    [d, s]/[d, t] layout by PE identity-transposes emitted on demand one
    chunk ahead (PSUM staging tiles borrow the score pool's slots).
  * Scores are computed TRANSPOSED: st[s_block=128, q_chunk<=512] =
    K_b^T-stationary x Q^T-moving; fp32 PSUM. Per-block causal trim
    (each block of a pair starts at its own diagonal offset; the skipped
    region of the second block is never read downstream).
  * Softmax exp is SPLIT across two engines (exp on ScalarE alone is an
    82us serial bottleneck at 1 elem/cycle/lane):
      - 3/4 of block-pairs: ScalarE LUT exp (exact, ~2ulp), fp16 out.
      - 1/4 of block-pairs: DVE Schraudolph exp -- one tensor_scalar
        i16 = rne(score * SCALE*log2e*1024 + (15360-57.75)) whose int16
        bit pattern IS fp16 2^t with linearly-interpolated mantissa.
        The 57.75 bias centers the mean ratio at 1 so mixing with exact
        exp stays unbiased. Measured end-to-end rel err 4.3e-3 (gate
        2e-2); the fraction is a speed/accuracy dial (1.4e-2 at 100%).
  * Causal mask: GPSIMD affine_select zeroes the s>q triangle of the
    diagonal prob tiles after exp.
  * PV: prob block [s,q-tile] is the STATIONARY operand, moving operand
    is [V_b | ones] [s, 129] fp16: accumulates [q, 128 out + 1 denom]
    in PSUM over s blocks -- the softmax denominator comes for free.
  * Software pipeline: QK runs TWO pairs ahead of PV (sc_psum bufs=3,
    6 banks) so the cross-engine exp latency is hidden; exp is emitted
    before the previous pair's PV/finalize so DVE exps don't queue
    behind finalize muls in the in-order DVE FIFO.
  * PV accumulators pack 2 tlocs per PSUM bank; only the first MM
    touching a bank carries start=True (start clears the WHOLE bank,
    per-element has_written handles overwrite-vs-accumulate after).
  * Finalize: DVE reciprocal of denom pairs + per-partition scalar
    multiply to fp32 SBUF, DMA out per chunk.
"""

import math

import numpy as np

import concourse.bass as bass
import concourse.tile as tile
from concourse import bacc, mybir
from concourse.masks import make_identity

P = 128
F32 = mybir.dt.float32
F16 = mybir.dt.float16
I16 = mybir.dt.int16
EXP = mybir.ActivationFunctionType.Exp

T_FULL = 2048
S_FULL = 2048
NH = 32
NKV = 8
D = 128
HQ = NH // NKV  # q heads per kv head (= per core)
N_CORES = 8

# Schraudolph fraction (pairs sent to DVE) and bias constant.
DVE_NUM, DVE_DEN = 1, 4
SCH_C = 57.75


def _attention_body(tc, T, S, HQ, D, chunk):
    nc = tc.nc
    NT = T // P          # q tiles
    NB = S // P          # s blocks
    TPC = chunk // P     # q tiles per chunk
    NCH = T // chunk     # chunks
    assert TPC % 2 == 0 and T % chunk == 0 and S == T
    SCALE = 1.0 / math.sqrt(D)
    LOG2E = math.log2(math.e)
    A_SCH = SCALE * LOG2E * 1024.0
    B_SCH = 15360.0 - SCH_C

    q = nc.dram_tensor("q", [T, HQ, D], F32, kind="ExternalInput").ap()
    k = nc.dram_tensor("k", [S, D], F32, kind="ExternalInput").ap()
    v = nc.dram_tensor("v", [S, D], F32, kind="ExternalInput").ap()
    out = nc.dram_tensor("out", [T, HQ, D], F32, kind="ExternalOutput").ap()

    from contextlib import ExitStack

    with ExitStack() as ctx:
        consts = ctx.enter_context(tc.tile_pool(name="consts", bufs=1))
        qT_pool = ctx.enter_context(tc.tile_pool(name="qT", bufs=4))
        q32_pool = ctx.enter_context(tc.tile_pool(name="q32", bufs=3))
        q16_pool = ctx.enter_context(tc.tile_pool(name="q16", bufs=3))
        et_pool = ctx.enter_context(tc.tile_pool(name="et", bufs=8))
        osb_pool = ctx.enter_context(tc.tile_pool(name="osb", bufs=4))
        rec_pool = ctx.enter_context(tc.tile_pool(name="rec", bufs=8))
        sc_psum = ctx.enter_context(tc.tile_pool(name="sc", bufs=3, space="PSUM"))
        pv_psum = ctx.enter_context(tc.tile_pool(name="pv", bufs=2, space="PSUM"))

        ident = consts.tile([P, P], F16)

        # ---- startup loads: dispatch everything with no deps first ----
        k_nat32 = consts.tile([P, NB, P], F32)
        k_nat = consts.tile([P, NB, P], F16)
        kT = consts.tile([P, NB * P], F16)
        k_r = k.rearrange("(b p) d -> p b d", p=P)
        v_sb = consts.tile([P, NB, P + 1], F16)  # [s_in_block, b, d|ones]
        v_nat32 = consts.tile([P, NB, P], F32)
        v_r = v.rearrange("(b p) d -> p b d", p=P)

        qTs = {}
        q32s = {}
        q16s = {}

        def q_tiles(h):
            if h not in qTs:
                qTs[h] = qT_pool.tile([P, T], F16, name=f"qT{h}", tag="qT")
                q32s[h] = q32_pool.tile([P, NT, P], F32, name=f"q32_{h}", tag="q32")
                q16s[h] = q16_pool.tile([P, NT, P], F16, name=f"q16_{h}", tag="q16")
            return qTs[h], q32s[h], q16s[h]

        def emit_q_load(h, g0, g1):
            # load + cast q tiles [g0*4, g1*4) of head h
            _, q32, q16 = q_tiles(h)
            t0, t1 = 4 * g0, 4 * g1
            q_rh = q[:, h, :].rearrange("(t p) d -> p t d", p=P)
            nc.sync.dma_start(out=q32[:, t0:t1, :], in_=q_rh[:, t0:t1, :])
            nc.vector.tensor_copy(q16[:, t0:t1, :], q32[:, t0:t1, :])

        q_loaded = set()

        def emit_q_load_full(h):
            if h in q_loaded:
                return
            q_loaded.add(h)
            q_tiles(h)
            emit_q_load(h, 0, 2)
            emit_q_load(h, 2, 4)

        # startup loads in NEED order, fine-grained so DMA bandwidth goes
        # to the first chunks' data first: chunk (0,c) needs k/q0 group c
        # and PV needs v blocks progressively.
        def k_load(g):
            bg = 4 * g
            nc.sync.dma_start(out=k_nat32[:, bg : bg + 4, :], in_=k_r[:, bg : bg + 4, :])
            nc.vector.tensor_copy(k_nat[:, bg : bg + 4, :], k_nat32[:, bg : bg + 4, :])

        def v_load(g):
            bg = 4 * g
            nc.sync.dma_start(out=v_nat32[:, bg : bg + 4, :], in_=v_r[:, bg : bg + 4, :])
            nc.vector.tensor_copy(v_sb[:, bg : bg + 4, 0:P], v_nat32[:, bg : bg + 4, :])

        k_load(0)
        emit_q_load(0, 0, 1)
        make_identity(nc, ident)
        v_load(0)
        emit_q_load(0, 1, 2)
        k_load(1)
        v_load(1)
        nc.vector.memset(v_sb[:, :, P : P + 1], 1.0)
        emit_q_load(0, 2, 3)
        k_load(2)
        v_load(2)
        emit_q_load(0, 3, 4)
        k_load(3)
        v_load(3)

        # ---- PE warmup: dummy matmuls (transpose-mode doesn't count as
        # PE-busy for the HAM) while the first loads are in flight trip
        # the HAM activity window so the first real matmuls run at 2.4GHz
        # instead of the cold 1.2GHz half-rate.
        warm_tp = sc_psum.tile([P, 4 * P], F32, tag="sc")
        for i in range(16):
            nc.tensor.matmul(
                warm_tp[:, (i % 4) * P : (i % 4 + 1) * P],
                lhsT=ident,
                rhs=ident,
                start=True,
                stop=True,
            )

        # ---- PE identity-transpose path (K and head 0) ----
        def pe_transpose(dst, src_tiles):
            # src_tiles: list of [P, P] f16 APs; dst: [P, 4*P] slice of kT/qT
            tp = sc_psum.tile([P, 4 * P], F16, tag="sc")
            for j, s_ap in enumerate(src_tiles):
                nc.tensor.transpose(tp[:, j * P : (j + 1) * P], s_ap, ident)
            nc.vector.tensor_copy(dst, tp)

        k_tp_done = set()

        def emit_ktp(g):
            if g in k_tp_done:
                return
            k_tp_done.add(g)
            bg = 4 * g
            pe_transpose(
                kT[:, bg * P : (bg + 4) * P],
                [k_nat[:, bg + j, :] for j in range(4)],
            )

        q_tp_done = set()

        def emit_qtp(h, c):
            if (h, c) in q_tp_done:
                return
            q_tp_done.add((h, c))
            qT, _, q16 = q_tiles(h)
            pe_transpose(
                qT[:, c * chunk : (c + 1) * chunk],
                [q16[:, c * TPC + j, :] for j in range(TPC)],
            )

        emit_ktp(0)
        emit_qtp(0, 0)

        # head 1 loads now (after startup loads); transposes on demand.
        emit_q_load_full(1)

        schedule = []
        for h in range(HQ):
            for cc in range(NCH):
                schedule.append((h, cc))

        # Bresenham assignment of pairs to the DVE Schraudolph path
        dve_pairs = set()
        acc = 0
        for h in range(HQ):
            for cc in range(NCH):
                for b0 in range(0, TPC * (cc + 1), 2):
                    acc += DVE_NUM
                    if acc >= DVE_DEN:
                        acc -= DVE_DEN
                        dve_pairs.add((h, cc, b0))

        def emit_prefetch(idx):
            h, c = schedule[idx]
            if c + 1 < NCH:
                if h == 0:
                    emit_ktp(c + 1)
                emit_qtp(h, c + 1)
            elif h + 1 < HQ:
                emit_qtp(h + 1, 0)
            if c == 0 and h + 2 < HQ:
                emit_q_load_full(h + 2)

        chunk_state = {}

        def get_state(idx, h, c):
            if idx not in chunk_state:
                chunk_state[idx] = {
                    "pvs": [
                        pv_psum.tile(
                            [P, 2, 132], F32, name=f"pv{idx}_{g}", tag="pv"
                        )
                        for g in range(TPC // 2)
                    ],
                    "osb": osb_pool.tile(
                        [P, TPC, P], F32, name=f"osb{idx}", tag="osb"
                    ),
                    "started": set(),
                }
            return chunk_state[idx]

        def emit_qk(idx, h, c, b0):
            qT = qTs[h]
            sc = sc_psum.tile([P, 2 * chunk], F32, name=f"sc{idx}_{b0}", tag="sc")
            for i, b in enumerate((b0, b0 + 1)):
                joff = max(0, b - c * TPC) * P
                nc.tensor.matmul(
                    sc[:, i * chunk + joff : (i + 1) * chunk],
                    lhsT=kT[:, b * P : (b + 1) * P],
                    rhs=qT[:, c * chunk + joff : (c + 1) * chunk],
                    start=True,
                    stop=True,
                )
            return sc

        def emit_exp_mask(idx, h, c, b0, sc):
            pair = (b0, b0 + 1)
            et = et_pool.tile([P, 2 * chunk], F16, name=f"et{idx}_{b0}", tag="et")
            joff0 = max(0, b0 - c * TPC) * P
            rng = slice(joff0, 2 * chunk)
            if (h, c, b0) in dve_pairs:
                nc.vector.tensor_scalar(
                    out=et[:, rng].bitcast(I16),
                    in0=sc[:, rng],
                    scalar1=A_SCH,
                    scalar2=B_SCH,
                    op0=mybir.AluOpType.mult,
                    op1=mybir.AluOpType.add,
                )
            else:
                nc.scalar.activation(et[:, rng], sc[:, rng], EXP, scale=SCALE)
            if b0 >= c * TPC:
                for i, b in enumerate(pair):
                    j = b - c * TPC
                    dsl = et[:, i * chunk + j * P : i * chunk + (j + 1) * P]
                    nc.gpsimd.affine_select(
                        out=dsl,
                        in_=dsl,
                        pattern=[[1, P]],
                        compare_op=mybir.AluOpType.is_ge,
                        fill=0.0,
                        base=0,
                        channel_multiplier=-1,
                    )
            return et

        def emit_pv(idx, h, c, b0, et):
            st = get_state(idx, h, c)
            work = []
            for i, b in enumerate((b0, b0 + 1)):
                j = b - c * TPC
                for tloc in range(max(0, j), TPC):
                    work.append((i, b, tloc, tloc == j))
            if b0 == 0:
                # first pair of a chunk: run bank-group-0 tlocs first --
                # group 1's bank is freed by the previous chunk's LAST
                # finalize, so giving it ~4 MMs of slack hides the handoff
                work.sort(key=lambda w: (w[3], w[2] // 2, w[0]))
            else:
                work.sort(key=lambda w: w[3])  # diagonal-tile PV last
            for i, b, tloc, _ in work:
                t = c * TPC + tloc
                g = tloc // 2
                first_touch = g not in st["started"]
                st["started"].add(g)
                nc.tensor.matmul(
                    st["pvs"][g][:, tloc % 2, 0 : P + 1],
                    lhsT=et[:, i * chunk + tloc * P : i * chunk + (tloc + 1) * P],
                    rhs=v_sb[:, b, :],
                    start=first_touch,
                    stop=(b == t),
                )

        def emit_finalize(idx, h, c, b0):
            st = chunk_state[idx]
            for b in (b0, b0 + 1):
                tloc = b - c * TPC
                if tloc < 0 or tloc % 2 != 0:
                    continue
                g = tloc // 2
                pv = st["pvs"][g]
                rec = rec_pool.tile(
                    [P, 2, 1], F32, name=f"rec{idx}_{g}", tag="rec"
                )
                nc.vector.reciprocal(rec, pv[:, :, P : P + 1])
                for j in range(2):
                    nc.vector.tensor_scalar_mul(
                        st["osb"][:, 2 * g + j, :], pv[:, j, 0:P], rec[:, j, :]
                    )

        def flush(entry):
            idx, h, c, b0, last, et = entry
            emit_pv(idx, h, c, b0, et)
            emit_finalize(idx, h, c, b0)
            if b0 == 0:
                emit_prefetch(idx)
            if last:
                nc.sync.dma_start(
                    out=out[c * chunk : (c + 1) * chunk, h, :].rearrange(
                        "(t p) d -> p t d", p=P
                    ),
                    in_=chunk_state[idx]["osb"],
                )
                del chunk_state[idx]

        # one flat software-pipelined stream over every (chunk, pair)
        stream = []
        for idx, (h, c) in enumerate(schedule):
            nblocks = TPC * (c + 1)
            for b0 in range(0, nblocks, 2):
                stream.append((idx, h, c, b0, b0 == nblocks - 2))

        # QK runs TWO pairs ahead of PV; exp emitted before the previous
        # pair's flush so DVE exps don't queue behind finalize muls.
        scs = {}

        def emit_qk_for(j):
            idx, h, c, b0, last = stream[j]
            # safety net for short chunks: the flush-prefetch may not
            # have emitted the PE transposes this QK reads yet
            for g in range((b0 + 1) // 4 + 1):
                emit_ktp(g)
            emit_qtp(h, c)
            get_state(idx, h, c)
            scs[j] = emit_qk(idx, h, c, b0)

        emit_qk_for(0)
        emit_qk_for(1)
        prev = None
        for j, (idx, h, c, b0, last) in enumerate(stream):
            if j + 2 < len(stream):
                emit_qk_for(j + 2)
            et = emit_exp_mask(idx, h, c, b0, scs.pop(j))
            if prev is not None:
                flush(prev)
            prev = (idx, h, c, b0, last, et)
        flush(prev)


def build_nc(T=T_FULL, S=S_FULL, HQ=HQ, D=D, chunk=512):
    nc = bacc.Bacc(
        "TRN2", target_bir_lowering=False, debug=False, enable_asserts=False
    )
    with tile.TileContext(nc) as tc:
        _attention_body(tc, T, S, HQ, D, chunk)
    nc.compile()
    return nc


_NC_CACHE = {}


def _get_nc():
    if "nc" not in _NC_CACHE:
        _NC_CACHE["nc"] = build_nc()
    return _NC_CACHE["nc"]


def kernel(q, k, v):
    """Full-problem entry point: q [2048,32,128], k/v [2048,8,128] f32."""
    from concourse.bass_utils import run_bass_kernel_spmd

    q = np.asarray(q, dtype=np.float32)
    k = np.asarray(k, dtype=np.float32)
    v = np.asarray(v, dtype=np.float32)

    nc = _get_nc()
    in_maps = []
    for i in range(N_CORES):
        in_maps.append(
            {
                "q": np.ascontiguousarray(q[:, HQ * i : HQ * (i + 1), :]),
                "k": np.ascontiguousarray(k[:, i, :]),
                "v": np.ascontiguousarray(v[:, i, :]),
            }
        )
    res = run_bass_kernel_spmd(nc, in_maps, core_ids=list(range(N_CORES)))
    out = np.empty((T_FULL, NH, D), dtype=np.float32)
    for i in range(N_CORES):
        out[:, HQ * i : HQ * (i + 1), :] = res.results[i]["out"]
    return out
